# revision 1
# baseline (speedup 1.0000x reference)
"""Causal multi-head self-attention (B=2, S=2048, D=768, H=12) on 8 TRN2 NeuronCores.

Sharding: core c = (batch b=c//4, head-group hg=c%4 of 3 heads).
Each core computes Q/K/V for its 3 heads, causal attention, and the partial
output projection sum_h out_h @ Wo[:, h]^T -> (S, D). Host sums the 4
head-group partials per batch (the unshard step).

On-core dataflow (transposed (feature, seq) layout, f32r matmuls):
  A) QKV^T: psum[m, s] += WcatT[i, m].T @ XT[i, s]   (i outer-of-sc so one
     stationary serves 4 moving matmuls; starts as soon as xt chunk 0 lands)
  B) V natural: PE-transpose V^T tiles -> V' = [V | ones] per k-tile
  C) per head, per q-half qp (2 q-chunks of 512), per k-tile t:
       scoresT[k, q] = KT[:,t].T @ QT   (only causally-valid halves)
       additive -30000 mask on the diagonal half, exp on ACT -> f32r,
       PV: pout[qc] += V'[t].T @ expT   (65 rows: 64 data + denominator)
     then per qc: recip(den) -> broadcast -> numerator * recip -> outcatT
  D) projection: psum[q, j] += outcatT[h, q].T @ WoT[h, j]; copy; DMA out.

PSUM budget in C: score tiles (128,1024)=2 banks x3 bufs + 2 pout banks = 8.
"""

import numpy as np
from contextlib import ExitStack

import concourse.bass as bass
import concourse.tile as tile
from concourse import bacc, mybir
from concourse import bass_utils

F32 = mybir.dt.float32
F32R = mybir.dt.float32r
BF16 = mybir.dt.bfloat16
FP16 = mybir.dt.float16
AF = mybir.ActivationFunctionType

B, S, D, H = 2, 2048, 768, 12
DK = 64
HPC = 3            # heads per core
NCORES = 8
NI = D // 128      # 6 input-feature chunks
NM = 5             # output m-chunks of 128 (640 rows incl. 64 pad)
NT = S // 128      # 16 k-tiles
NQC = S // 512     # 4 q-chunks
MASK_NEG = -30000.0

# per-local-head (base_partition, m_chunk) in the QKVT buffer
QPOS = [(0, 0), (64, 0), (0, 2)]
KPOS = [(0, 1), (64, 1), (0, 3)]
VPOS = [(64, 2), (64, 3), (0, 4)]

_NC_CACHE = {}


def _enable_ldw_opt():
    """Let walrus dedupe back-to-back identical weight loads (verified
    bit-identical output on this kernel; saves ~1/3 of LDWEIGHTS)."""
    if getattr(bass_utils.run_command, "_ldw_patched", False):
        return
    orig = bass_utils.run_command

    def patched(argv, **kw):
        argv = ["--enable-ldw-opt=true" if a == "--enable-ldw-opt=false" else a
                for a in argv]
        return orig(argv, **kw)

    patched._ldw_patched = True
    bass_utils.run_command = patched


def build_nc(dbg=False):
    key = ("nc", dbg)
    if key in _NC_CACHE:
        return _NC_CACHE[key]
    # ldw-opt incompatible with fp16 matmul explicit ldweights
    nc = bacc.Bacc("TRN2", target_bir_lowering=False, debug=False,
                   num_devices=NCORES)

    xt_d = nc.dram_tensor("xt", [NI, 128, S], F32R, kind="ExternalInput").ap()
    wcat_d = nc.dram_tensor("wcat", [NI, 128, NM * 128], F32R, kind="ExternalInput").ap()
    wot_d = nc.dram_tensor("wot", [2, 128, D], FP16, kind="ExternalInput").ap()
    mask_d = nc.dram_tensor("mask", [128, 128], F32, kind="ExternalInput").ap()
    id_d = nc.dram_tensor("ident", [128, 128], F32R, kind="ExternalInput").ap()
    ones_d = nc.dram_tensor("vones", [128, HPC * NT], FP16, kind="ExternalInput").ap()
    out_d = nc.dram_tensor("out", [S, D], F32, kind="ExternalOutput").ap()
    if dbg:
        qkv_dbg = nc.dram_tensor("qkv_dbg", [128, NM, S], F32, kind="ExternalOutput").ap()
        vp_dbg = nc.dram_tensor("vp_dbg", [128, HPC, NT, DK + 1], F32, kind="ExternalOutput").ap()
        oct_dbg = nc.dram_tensor("oct_dbg", [DK, HPC, S], F32, kind="ExternalOutput").ap()

    with tile.TileContext(nc) as tc, ExitStack() as ctx:
        const = ctx.enter_context(tc.tile_pool(name="const", bufs=1))

        # persistent SBUF buffers
        xt = const.tile([128, NI, S], F32R)             # X^T
        wcat = const.tile([128, NI, NM * 128], F32R)    # W^T (QKV packed)
        wot = const.tile([128, 2, D], FP16)             # Wo^T [h0;h1],[h2;pad]
        maskb = const.tile([128, 128], F32)             # diag causal bias tile
        ident = const.tile([128, 128], F32R)
        qkvt = const.tile([128, NM, S], F32R)           # Q^T/K^T/V^T packed
        vp = const.tile([128, HPC, NT, DK + 1], FP16)   # V' = [V | ones]
        oct_ = const.tile([128, 2, S], FP16)            # packed out^T [h0;h1],[h2]
        qk16 = const.tile([128, 4, S], FP16)            # fp16 Q/K for attention

        # priority: what phase A's first accumulation chains touch first
        for i in range(NI):
            nc.sync.dma_start(wcat[:, i, 0:128], wcat_d[i][:, 0:128])
        for sh in range(4):
            for i in range(NI):
                nc.sync.dma_start(xt[:, i, sh * 512:(sh + 1) * 512],
                                  xt_d[i][:, sh * 512:(sh + 1) * 512])
        for i in range(NI):
            nc.sync.dma_start(wcat[:, i, 128:NM * 128], wcat_d[i][:, 128:NM * 128])
        nc.sync.dma_start(ident[:], id_d)
        nc.sync.dma_start(vp[:, :, :, DK:DK + 1],
                          ones_d.rearrange("p (h t) -> p h t", h=HPC))
        nc.sync.dma_start(maskb[:], mask_d)
        nc.sync.dma_start(wot[:], wot_d.rearrange("c p f -> p c f"))

        # ---- Phase A: QKV^T projection; Phase B: V transposes (shared pool)
        with tc.tile_pool(name="ps_ab", bufs=4, space="PSUM") as ps_ab:
            for m in range(NM):
                pqs = [ps_ab.tile([128, 512], F32, tag="proj", name=f"pq{m}_{sc}")
                       for sc in range(NQC)]
                for i in range(NI):
                    for sc in range(NQC):
                        nc.tensor.matmul(
                            pqs[sc][:],
                            wcat[:, i, m * 128:(m + 1) * 128],
                            xt[:, i, sc * 512:(sc + 1) * 512],
                            start=(i == 0), stop=(i == NI - 1),
                        )
                for sc in range(NQC):
                    nc.vector.tensor_copy(
                        qkvt[:, m, sc * 512:(sc + 1) * 512], pqs[sc][:])
                # fp16 shadow of Q/K rows for the attention core
                if m <= 1:
                    for sc in range(NQC):
                        nc.vector.tensor_copy(
                            qk16[:, m, sc * 512:(sc + 1) * 512],
                            pqs[sc][:])
                elif m <= 3:
                    for sc in range(NQC):
                        nc.vector.tensor_copy(
                            qk16[0:DK, m, sc * 512:(sc + 1) * 512],
                            pqs[sc][0:DK, :])

            for h in range(HPC):
                vb, vchunk = VPOS[h]
                for t in range(NT):
                    ptr = ps_ab.tile([128, DK], F32R, tag="tr", bufs=4,
                                     name=f"tr{h}_{t}")
                    nc.tensor.transpose(
                        ptr[:],
                        qkvt[vb:vb + DK, vchunk, t * 128:(t + 1) * 128],
                        ident[vb:vb + DK, vb:vb + DK],
                    )
                    nc.vector.tensor_copy(vp[:, h, t, 0:DK], ptr[:])

        # ---- Phase C: attention per head, q-half outer (pscr triple-buffered)
        with tc.tile_pool(name="ps_s", bufs=3, space="PSUM") as ps_s, \
             tc.tile_pool(name="ps_o", bufs=2, space="PSUM") as ps_o, \
             tc.tile_pool(name="sb_exp", bufs=6) as sb_exp, \
             tc.tile_pool(name="sb_div", bufs=3) as sb_div:
            for h in range(HPC):
                qb, qchunk = QPOS[h]
                kb, kchunk = KPOS[h]
                pouts = {}

                def score_step(qp, t):
                    qcs = (2 * qp, 2 * qp + 1)
                    qc_lo = t // 4
                    off = 128 * (t % 4)   # diag col offset inside qc_lo's half
                    pscr = ps_s.tile([128, 1024], F32, tag="scr",
                                     name=f"sc{h}_{qp}_{t}")
                    for half, qc in enumerate(qcs):
                        if qc < qc_lo:
                            continue
                        cs = off if qc == qc_lo else 0  # skip fully-masked cols
                        nc.tensor.matmul(
                            pscr[:, half * 512 + cs:(half + 1) * 512],
                            qk16[kb:kb + DK, kchunk, t * 128:(t + 1) * 128],
                            qk16[qb:qb + DK, qchunk,
                                 qc * 512 + cs:(qc + 1) * 512],
                            start=True, stop=True,
                        )
                    if qc_lo in qcs:  # mask only the 128-wide diagonal window
                        half = qc_lo - 2 * qp
                        nc.vector.tensor_add(
                            pscr[:, half * 512 + off:half * 512 + off + 128],
                            pscr[:, half * 512 + off:half * 512 + off + 128],
                            maskb[:, 0:128],
                        )
                    lo = (512 if qc_lo == qcs[1] else 0) + \
                         (off if qc_lo in qcs else 0)
                    expt = sb_exp.tile([128, 1024], FP16, tag="exp",
                                       name=f"ex{h}_{qp}_{t}")
                    nc.scalar.activation(expt[:, lo:1024], pscr[:, lo:1024],
                                         AF.Exp)
                    return expt

                def pv_step(qp, t, expt):
                    qcs = (2 * qp, 2 * qp + 1)
                    qc_lo = t // 4
                    off = 128 * (t % 4)
                    for half, qc in enumerate(qcs):
                        if qc < qc_lo:
                            continue
                        cs = off if qc == qc_lo else 0
                        nc.tensor.matmul(
                            pouts[qc][:, cs:512],
                            vp[:, h, t, :],
                            expt[:, half * 512 + cs:(half + 1) * 512],
                            start=(t == 0), stop=(t == 4 * qc + 3),
                        )

                def divide(qc):
                    # evict the finished chain at once so its PSUM bank frees
                    # immediately; the slow recip/divide runs off the copy
                    nout = sb_div.tile([DK + 1, 512], F32, tag="nout",
                                       name=f"no{h}_{qc}")
                    nc.vector.tensor_copy(nout[:], pouts[qc][:])
                    # spread the 512-wide den row over 64 partitions so the
                    # expensive reciprocal runs 64 lanes wide, not 1
                    rsp = sb_div.tile([DK, 8], F32, tag="rsp",
                                      name=f"rsp{h}_{qc}")
                    nc.sync.dma_start(rsp[:], nout[DK:DK + 1, :])
                    rcs = sb_div.tile([DK, 8], F32, tag="rcs",
                                      name=f"rcs{h}_{qc}")
                    nc.vector.reciprocal(rcs[:], rsp[:])
                    rc0 = sb_div.tile([1, 512], F32, tag="rc0",
                                      name=f"rc0{h}_{qc}")
                    nc.sync.dma_start(rc0[:], rcs[:])
                    rb = sb_div.tile([DK, 512], F32, tag="rb",
                                     name=f"rb{h}_{qc}")
                    nc.gpsimd.partition_broadcast(rb[:], rc0[:])
                    if h == 1:
                        # h1 lands at partitions 64-127: shift via SBUF DMA
                        tmp = sb_div.tile([DK, 512], FP16, tag="tmp",
                                          name=f"tmp{h}_{qc}")
                        nc.vector.tensor_mul(tmp[:], nout[0:DK, :], rb[:])
                        nc.sync.dma_start(
                            oct_[DK:128, 0, qc * 512:(qc + 1) * 512], tmp[:])
                    else:
                        nc.vector.tensor_mul(
                            oct_[0:DK, h // 2, qc * 512:(qc + 1) * 512],
                            nout[0:DK, :], rb[:],
                        )

                # pair the k-tiles: two same-geometry score LDW+MMs
                # back-to-back, then two same-geometry PV LDW+MMs — halves
                # PE stationary-geometry switches (measured ~2x matmul cost
                # per switch in isolation)
                for qp in range(2):
                    for qc in (2 * qp, 2 * qp + 1):
                        pouts[qc] = ps_o.tile([DK + 1, 512], F32, tag="pout",
                                              name=f"po{h}_{qc}")
                    for t0 in range(0, 4 * (2 * qp + 1) + 4, 2):
                        e0 = score_step(qp, t0)
                        e1 = score_step(qp, t0 + 1)
                        pv_step(qp, t0, e0)
                        pv_step(qp, t0 + 1, e1)
                        if t0 + 1 == 4 * (2 * qp) + 3:
                            divide(2 * qp)      # low chain done: free its bank
                    divide(2 * qp + 1)

            # ---- output projection, reusing the attention pools' slots
            for qt in range(NT):
                pp = ps_s.tile([128, D], F32, tag="scr", name=f"pp{qt}")
                for c, kk in ((0, 128), (1, DK)):
                    for js, je in ((0, 512), (512, D)):
                        nc.tensor.matmul(
                            pp[:, js:je],
                            oct_[0:kk, c, qt * 128:(qt + 1) * 128],
                            wot[0:kk, c, js:je],
                            start=(c == 0), stop=(c == 1),
                        )
                ot = sb_exp.tile([128, D], F32, tag="exp", name=f"ot{qt}")
                nc.vector.tensor_copy(ot[:], pp[:])
                nc.sync.dma_start(out_d[qt * 128:(qt + 1) * 128, :], ot[:])

        if dbg:
            nc.sync.dma_start(qkv_dbg, qkvt[:].bitcast(F32))
            nc.sync.dma_start(vp_dbg, vp[:].bitcast(F32))
            nc.sync.dma_start(oct_dbg, oct_[:].bitcast(F32))


    nc.compile()
    _NC_CACHE[key] = nc
    return nc


def make_in_maps(X, Wq, Wk, Wv, Wo):
    X = np.ascontiguousarray(np.asarray(X, dtype=np.float32))
    Wq = np.asarray(Wq, dtype=np.float32)
    Wk = np.asarray(Wk, dtype=np.float32)
    Wv = np.asarray(Wv, dtype=np.float32)
    Wo = np.asarray(Wo, dtype=np.float32)

    # causal additive-bias tiles: keep q >= k; rows=k (p), cols=q (f),
    # diagonal offset delta = 128*di: keep iff f >= p + delta
    p = np.arange(128)[:, None]
    f = np.arange(512)[None, :]
    mask = np.where(f[:, :128] >= p, 0.0, MASK_NEG).astype(np.float32)
    ident = np.eye(128, dtype=np.float32)
    vones = np.ones((128, HPC * NT), dtype=np.float16)

    in_maps = []
    for c in range(NCORES):
        b, hg = c // 4, c % 4
        gh = [hg * HPC + l for l in range(HPC)]
        q = [Wq[g * DK:(g + 1) * DK, :] / 8.0 for g in gh]
        k = [Wk[g * DK:(g + 1) * DK, :] for g in gh]
        v = [Wv[g * DK:(g + 1) * DK, :] for g in gh]
        wcat_rows = np.vstack([
            q[0], q[1], k[0], k[1], q[2], v[0], k[2], v[1], v[2],
            np.zeros((DK, D), dtype=np.float32),
        ])                                            # (640, 768)
        wcat = np.ascontiguousarray(wcat_rows.T.reshape(NI, 128, NM * 128))
        w0, w1, w2 = (Wo[:, g * DK:(g + 1) * DK].T for g in gh)
        wot = np.ascontiguousarray(np.stack([
            np.vstack([w0, w1]),
            np.vstack([w2, np.zeros((DK, D), dtype=np.float32)]),
        ]).astype(np.float16))                                     # (2, 128, 768)
        xt = np.ascontiguousarray(X[b].T.reshape(NI, 128, S))
        in_maps.append({
            "xt": xt, "wcat": wcat, "wot": wot,
            "mask": mask, "ident": ident, "vones": vones,
        })
    return in_maps


def _run(in_maps, trace=False, trace_cores=None):
    nc = build_nc()
    return bass_utils.run_bass_kernel_spmd(
        nc, in_maps, core_ids=list(range(NCORES)),
        trace=trace, trace_cores=trace_cores,
    )


def kernel(X, Wq, Wk, Wv, Wo):
    in_maps = make_in_maps(X, Wq, Wk, Wv, Wo)
    res = _run(in_maps, trace=False)
    out = np.zeros((B, S, D), dtype=np.float32)
    for c in range(NCORES):
        out[c // 4] += res.results[c]["out"]
    return out

